# revision 1
# baseline (speedup 1.0000x reference)
# nn_ClustGeoEdgeEncoder on 8 Trainium2 NeuronCores.
#
# Data-parallel over the edge dimension: edge_index is split into 8 shards
# of 1024 edges (one per core); data and clusts are replicated. Each core
# gathers its edges' cluster point sets, does the 128x128 cdist+argmin and
# feature build, and the full [8192, 19] output is reassembled on the host.
import sys
sys.path.insert(0, "/opt/trn_rl_repo")
import numpy as np

N_PTS, N_CLUSTS, P, E = 262144, 2048, 128, 8192
N_CORES = 8
E_CHUNK = 128  # edges per inner chunk (bounds the [e,128,128] intermediate)

_COMPILED = {}


def _build():
    import jax
    import jax.numpy as jnp
    from jax.sharding import Mesh, PartitionSpec
    from jax.experimental.shard_map import shard_map

    devices = jax.devices()[:N_CORES]
    mesh = Mesh(np.asarray(devices), ("core",))
    Ps = PartitionSpec

    def per_core(data, clusts, ei_loc):
        vox = data[:, :3].astype(jnp.float32)

        def chunk(ei_c):
            x1 = vox[clusts[ei_c[0]]]          # [c, P, 3]
            x2 = vox[clusts[ei_c[1]]]
            d2 = jnp.sum(
                (x1[:, :, None, :] - x2[:, None, :, :]) ** 2, axis=-1)
            imin = jnp.argmin(d2.reshape(d2.shape[0], -1), axis=1)
            i1, i2 = imin // P, imin % P
            eidx = jnp.arange(d2.shape[0])
            v1 = x1[eidx, i1]
            v2 = x2[eidx, i2]
            disp = v1 - v2
            lend = jnp.linalg.norm(disp, axis=-1, keepdims=True)
            safe = jnp.where(lend > 0, lend, 1.0)
            dispn = jnp.where(lend > 0, disp / safe, disp)
            B = (dispn[:, :, None] * dispn[:, None, :]).reshape(-1, 9)
            return jnp.concatenate([v1, v2, dispn, lend, B], axis=1)

        e_loc = E // N_CORES
        outs = []
        for s in range(0, e_loc, E_CHUNK):
            outs.append(chunk(ei_loc[:, s:s + E_CHUNK]))
        return jnp.concatenate(outs, axis=0)

    fn = shard_map(
        per_core, mesh=mesh,
        in_specs=(Ps(), Ps(), Ps(None, "core")),
        out_specs=Ps("core"),
        check_rep=False,
    )
    return jax.jit(fn)


def kernel(data, clusts, edge_index):
    import jax.numpy as jnp
    if "fn" not in _COMPILED:
        _COMPILED["fn"] = _build()
    fn = _COMPILED["fn"]
    data_j = jnp.asarray(np.asarray(data, dtype=np.float32))
    clusts_j = jnp.asarray(np.asarray(clusts, dtype=np.int32))
    ei_j = jnp.asarray(np.asarray(edge_index, dtype=np.int32))
    out = fn(data_j, clusts_j, ei_j)
    return np.asarray(out).astype(np.float32)


if __name__ == "__main__":
    rng = np.random.default_rng(0)
    data = (rng.standard_normal((N_PTS, 5)) * 100).astype(np.float32)
    clusts = rng.integers(0, N_PTS, size=(N_CLUSTS, P)).astype(np.int32)
    ei = rng.integers(0, N_CLUSTS, size=(2, E)).astype(np.int32)
    out = kernel(data, clusts, ei)
    print("out", out.shape, out.dtype)

